# revision 1
# baseline (speedup 1.0000x reference)
"""Trainium2 Bass kernel for DenseConv2d.

Conv2d: input (32,128,56,56) f32, weight (256,128,3,3) f32, bias (256,) f32,
stride 1, pad 1, dilation 1 -> output (32,256,56,56) f32.

Strategy: data-parallel over batch across 8 NeuronCores (4 images per core).
Per core the conv is computed as 9 accumulated matmuls (one per kernel tap)
into PSUM: out[co, pix] += W[kh,kw][ci,co].T @ x_pad[ci, shifted pix window].
Operands stream through the PE array as float32r (~1.1 cycles/row sustained).
Input is chunked (2 row-blocks + halo per DMA) on the scalar-engine HWDGE
queue so the first matmul starts as early as possible; output DMAs ride the
sync queue. A few warmup matmuls on scratch data run during the input DMA
wait to lift the PE HAM clock-gate to 2.4 GHz before real work arrives.
Layout prep (padding, channel-major transpose) is host-side numpy.
"""

import sys

if "/opt/trn_rl_repo" not in sys.path:
    sys.path.insert(0, "/opt/trn_rl_repo")

import numpy as np

N_CORES = 8
N, CI, H, W = 32, 128, 56, 56
CO, KH, KW = 256, 3, 3
NP_CORE = N // N_CORES          # images per core
HP, WP = H + 2, W + 2           # padded spatial dims
COT = CO // 128                 # out-channel tiles of 128
RB = 8                          # output rows per matmul block
NBLK = H // RB                  # row blocks per image
CHROWS = 2 * RB + 2             # input rows per chunk (2 blocks + halo)
NCH = 4                         # chunks per image (last one is short)
N_WARMUP = 5                    # PE warmup matmuls

_CACHE = {}


def _build_program():
    import concourse.mybir as mybir
    from concourse import bacc
    from concourse.tile import TileContext

    nc = bacc.Bacc(None, target_bir_lowering=False)

    x_d = nc.dram_tensor("x", [CI, NP_CORE, HP, WP], mybir.dt.float32r,
                         kind="ExternalInput")
    w_d = nc.dram_tensor("w", [CI, COT, KH * KW, 128], mybir.dt.float32r,
                         kind="ExternalInput")
    b_d = nc.dram_tensor("b2", [128, COT], mybir.dt.float32,
                         kind="ExternalInput")
    y_d = nc.dram_tensor("y", [COT, 128, NP_CORE, H, W], mybir.dt.float32,
                         kind="ExternalOutput")

    f32 = mybir.dt.float32
    f32r = mybir.dt.float32r

    with TileContext(nc) as tc:
        with (
            tc.tile_pool(name="xin", bufs=1) as xpool,
            tc.tile_pool(name="wpool", bufs=1) as wpool,
            tc.tile_pool(name="bpool", bufs=1) as bpool,
            tc.tile_pool(name="psum", bufs=8, space="PSUM") as ppool,
            tc.tile_pool(name="out", bufs=6) as opool,
        ):
            # PE warmup on scratch data, concurrent with the input DMAs,
            # so the HAM clock-gate is at 2.4 GHz when real matmuls start.
            scratch = xpool.tile([CI, RB * W], mybir.dt.bfloat16,
                                 tag="scratch")
            nc.vector.memset(scratch, 0.0)
            wups = ppool.tile([128, RB * W], f32, tag="ps")
            for _ in range(N_WARMUP):
                nc.tensor.matmul(wups, scratch[:, 0:128], scratch,
                                 start=True, stop=True)
            # Tiny-warmup tail (~53 ns each, ~1.6 us total): bridges the
            # PE-busy window from the big warmups to the first input
            # chunk's arrival, so HAM is at 2.4 GHz and the real stream
            # never starts cold.
            for _ in range(30):
                nc.tensor.matmul(wups[:, 0:64], scratch[:, 0:128],
                                 scratch[:, 0:64], start=True, stop=True)

            # Weights split by out-channel tile so the first matmul group
            # only waits for w[cot=0] + the first input chunk (~0.7 MB).
            wt = []
            for cot in range(COT):
                wtile = wpool.tile([CI, KH * KW, 128], f32r, tag=f"w{cot}")
                wt.append(wtile)
            bt = bpool.tile([128, COT], f32)

            def wslice(pos, cot):
                return wt[cot][:, pos, :]

            # Input chunks per image: (padded_row0, n_blocks). The first is
            # a single block so the very first matmul group's data arrives
            # fast; block b lives in chunk CHMAP[b] at local row CHLOC[b].
            CHUNKS = [(0, 1), (RB, 2), (3 * RB, 2), (5 * RB, 2)]
            CHMAP, CHLOC = {}, {}
            b = 0
            for ci_, (r0_, nb_) in enumerate(CHUNKS):
                for j in range(nb_):
                    CHMAP[b], CHLOC[b] = ci_, j * RB
                    b += 1
            xt = {}

            def x_chunk_dma(img, ch, eng):
                r0, nb = CHUNKS[ch]
                rows = min(nb * RB + 2, HP - r0)
                t = xpool.tile([CI, rows, WP], f32r, tag=f"x{img}_{ch}")
                eng.dma_start(out=t, in_=x_d[:, img, r0:r0 + rows, :])
                xt[img, ch] = t

            # Critical path: the first matmul group needs x0 chunk0 plus all
            # 9 taps of w[cot0]; spread those over both HWDGE queues.
            nc.scalar.dma_start(out=wt[0][:, 0:5, :], in_=w_d[:, 0, 0:5, :])
            x_chunk_dma(0, 0, nc.sync)
            nc.sync.dma_start(out=wt[0][:, 5:9, :], in_=w_d[:, 0, 5:9, :])
            x_chunk_dma(0, 1, nc.scalar)
            nc.sync.dma_start(out=wt[1], in_=w_d[:, 1, :, :])
            x_chunk_dma(0, 2, nc.scalar)
            x_chunk_dma(0, 3, nc.sync)
            nc.scalar.dma_start(out=bt, in_=b_d[:, :])
            for img in range(1, NP_CORE):
                for ch in range(len(CHUNKS)):
                    x_chunk_dma(img, ch, nc.scalar)

            for img in range(NP_CORE):
                for cot in range(COT):
                    for blk in range(NBLK):
                        ps = ppool.tile([128, RB, W], f32, tag="ps")
                        ch, r0 = CHMAP[blk], CHLOC[blk]
                        for pos in range(KH * KW):
                            kh, kw = divmod(pos, KW)
                            rhs = xt[img, ch][:, r0 + kh:r0 + kh + RB,
                                              kw:kw + W]
                            nc.tensor.matmul(
                                ps, wslice(pos, cot), rhs,
                                start=(pos == 0), stop=(pos == KH * KW - 1),
                            )
                        last = (img == NP_CORE - 1 and cot == COT - 1
                                and blk == NBLK - 1)
                        if last:
                            # Tail: ship the final block as two half-copies on
                            # both queues so the store pipeline drains sooner.
                            h = RB // 2
                            ot1 = opool.tile([128, h, W], f32, tag="ot1")
                            nc.vector.tensor_scalar_add(
                                ot1, ps[:, 0:h, :], bt[:, cot:cot + 1])
                            nc.sync.dma_start(
                                out=y_d[cot, :, img,
                                        blk * RB:blk * RB + h, :], in_=ot1)
                            ot2 = opool.tile([128, h, W], f32, tag="ot2")
                            nc.vector.tensor_scalar_add(
                                ot2, ps[:, h:RB, :], bt[:, cot:cot + 1])
                            nc.scalar.dma_start(
                                out=y_d[cot, :, img,
                                        blk * RB + h:blk * RB + RB, :],
                                in_=ot2)
                        else:
                            ot = opool.tile([128, RB, W], f32)
                            nc.vector.tensor_scalar_add(
                                ot, ps, bt[:, cot:cot + 1])
                            nc.sync.dma_start(
                                out=y_d[cot, :, img,
                                        blk * RB:blk * RB + RB, :], in_=ot)

    nc.compile()
    return nc


def prep_in_maps(input, weight, bias):
    """Host-side layout prep -> one in_map per core."""
    xp = np.pad(input, ((0, 0), (0, 0), (1, 1), (1, 1)))
    # weight [co, ci, kh, kw] -> [ci, cot, (kh kw), cop]
    wr = np.ascontiguousarray(
        weight.transpose(1, 2, 3, 0).reshape(CI, KH * KW, COT, 128)
        .transpose(0, 2, 1, 3))
    b2 = np.ascontiguousarray(bias.reshape(COT, 128).T)

    in_maps = []
    for c in range(N_CORES):
        xc = np.ascontiguousarray(
            xp[c * NP_CORE:(c + 1) * NP_CORE].transpose(1, 0, 2, 3))
        in_maps.append({"x": xc, "w": wr, "b2": b2})
    return in_maps


def kernel(input, weight, bias):
    input = np.asarray(input, dtype=np.float32)
    weight = np.asarray(weight, dtype=np.float32)
    bias = np.asarray(bias, dtype=np.float32)

    if "nc" not in _CACHE:
        _CACHE["nc"] = _build_program()
    nc = _CACHE["nc"]

    from concourse.bass_utils import run_bass_kernel_spmd

    in_maps = prep_in_maps(input, weight, bias)
    res = run_bass_kernel_spmd(nc, in_maps, core_ids=list(range(N_CORES)))

    out = np.empty((N, CO, H, W), dtype=np.float32)
    for c in range(N_CORES):
        y = res.results[c]["y"]  # [COT, 128, NP_CORE, H, W]
        out[c * NP_CORE:(c + 1) * NP_CORE] = (
            y.transpose(2, 0, 1, 3, 4).reshape(NP_CORE, CO, H, W))
    return out



# revision 2
# speedup vs baseline: 1.0795x; 1.0795x over previous
"""Trainium2 Bass kernel for DenseConv2d.

Conv2d: input (32,128,56,56) f32, weight (256,128,3,3) f32, bias (256,) f32,
stride 1, pad 1, dilation 1 -> output (32,256,56,56) f32.

Strategy: data-parallel over batch across 8 NeuronCores (4 images per core).
Per core the conv is computed as 9 accumulated matmuls (one per kernel tap)
into PSUM: out[co, pix] += W[kh,kw][ci,co].T @ x_pad[ci, shifted pix window].
Operands are cast host-side to bfloat16 so the PE array streams 1 column per
cycle (448-col matmul every ~189 ns warm) and weight loads ride the FWL fast
path; accumulation stays fp32 in PSUM. Input arrives as one 10-row chunk per
output row-block, interleaved across both HWDGE queues so the first matmul
group's data (x rows 0-9 + the 9 cot0 taps) lands as early as possible.
Warmup matmuls on scratch data bridge the DMA wait so the PE HAM clock-gate
is at 2.4 GHz when real work arrives. Output stores ride the sync queue;
the final block drains as four quarter-stores alternating across both queues
to shorten the tail. Layout prep (padding, channel-major transpose, bf16
cast) is host-side numpy.
"""

import sys

if "/opt/trn_rl_repo" not in sys.path:
    sys.path.insert(0, "/opt/trn_rl_repo")

import numpy as np

N_CORES = 8
N, CI, H, W = 32, 128, 56, 56
CO, KH, KW = 256, 3, 3
NP_CORE = N // N_CORES          # images per core
HP, WP = H + 2, W + 2           # padded spatial dims
COT = CO // 128                 # out-channel tiles of 128
RB = 8                          # output rows per matmul block
NBLK = H // RB                  # row blocks per image
CHROWS = RB + 2                 # input rows per chunk (1 block + halo)
N_WARM_BIG = 6                  # 448-col warmup matmuls (~373 ns cold each)
N_WARM_TINY = 14                # 64-col warmup matmuls (~53 ns each)

_CACHE = {}


def _build_program():
    import concourse.mybir as mybir
    from concourse import bacc
    from concourse.tile import TileContext

    nc = bacc.Bacc(None, target_bir_lowering=False)

    bf16 = mybir.dt.bfloat16
    f32 = mybir.dt.float32

    x_d = nc.dram_tensor("x", [CI, NP_CORE, HP, WP], bf16,
                         kind="ExternalInput")
    w_d = nc.dram_tensor("w", [CI, COT, KH * KW, 128], bf16,
                         kind="ExternalInput")
    b_d = nc.dram_tensor("b2", [128, COT], f32,
                         kind="ExternalInput")
    y_d = nc.dram_tensor("y", [COT, 128, NP_CORE, H, W], f32,
                         kind="ExternalOutput")

    with TileContext(nc) as tc:
        with (
            tc.tile_pool(name="xin", bufs=1) as xpool,
            tc.tile_pool(name="wpool", bufs=1) as wpool,
            tc.tile_pool(name="bpool", bufs=1) as bpool,
            tc.tile_pool(name="psum", bufs=8, space="PSUM") as ppool,
            tc.tile_pool(name="out", bufs=6) as opool,
        ):
            # PE warmup on scratch data, concurrent with the input DMAs,
            # so the HAM clock-gate is at 2.4 GHz when real matmuls start.
            scratch = xpool.tile([CI, RB * W], bf16, tag="scratch")
            nc.vector.memset(scratch, 0.0)
            wups = ppool.tile([128, RB * W], f32, tag="ps")
            for _ in range(N_WARM_BIG):
                nc.tensor.matmul(wups, scratch[:, 0:128], scratch,
                                 start=True, stop=True)
            # Tiny-warmup tail: bridges the PE-busy window from the big
            # warmups to the first input chunk's arrival at fine grain, so
            # the switch to real work costs at most one tiny matmul.
            for _ in range(N_WARM_TINY):
                nc.tensor.matmul(wups[:, 0:64], scratch[:, 0:128],
                                 scratch[:, 0:64], start=True, stop=True)

            wt = []
            for cot in range(COT):
                wtile = wpool.tile([CI, KH * KW, 128], bf16, tag=f"w{cot}")
                wt.append(wtile)
            bt = bpool.tile([128, COT], f32)

            # One 10-row chunk per (img, block): blk b needs padded rows
            # 8b .. 8b+9. Uniform size keeps the pipeline smooth and lets
            # img0's blocks arrive one by one right ahead of the stream.
            xt = {}

            def x_chunk_dma(img, blk, eng):
                t = xpool.tile([CI, CHROWS, WP], bf16, tag=f"x{img}_{blk}")
                eng.dma_start(out=t, in_=x_d[:, img, blk * RB:blk * RB + CHROWS, :])
                xt[img, blk] = t

            # Critical path: group (img0,cot0,blk0) needs x0b0 + taps 0..8
            # of w[cot0]. Interleave across both HWDGE queues so its pieces
            # land back-to-back; later chunks follow in first-use order.
            x_chunk_dma(0, 0, nc.sync)
            nc.scalar.dma_start(out=wt[0][:, 0:5, :], in_=w_d[:, 0, 0:5, :])
            nc.sync.dma_start(out=wt[0][:, 5:9, :], in_=w_d[:, 0, 5:9, :])
            x_chunk_dma(0, 1, nc.scalar)
            x_chunk_dma(0, 2, nc.sync)
            x_chunk_dma(0, 3, nc.scalar)
            x_chunk_dma(0, 4, nc.sync)
            x_chunk_dma(0, 5, nc.scalar)
            x_chunk_dma(0, 6, nc.sync)
            nc.scalar.dma_start(out=bt, in_=b_d[:, :])
            nc.sync.dma_start(out=wt[1], in_=w_d[:, 1, :, :])
            for img in range(1, NP_CORE):
                for blk in range(NBLK):
                    x_chunk_dma(img, blk, nc.scalar)

            for img in range(NP_CORE):
                for cot in range(COT):
                    for blk in range(NBLK):
                        ps = ppool.tile([128, RB, W], f32, tag="ps")
                        xc = xt[img, blk]
                        for pos in range(KH * KW):
                            kh, kw = divmod(pos, KW)
                            rhs = xc[:, kh:kh + RB, kw:kw + W]
                            nc.tensor.matmul(
                                ps, wt[cot][:, pos, :], rhs,
                                start=(pos == 0), stop=(pos == KH * KW - 1),
                            )
                        last = (img == NP_CORE - 1 and cot == COT - 1
                                and blk == NBLK - 1)
                        if last:
                            # Tail: drain the final block as four quarter
                            # copies alternating across both queues so the
                            # store pipeline empties as soon as possible.
                            q = RB // 4
                            for j in range(4):
                                eng = nc.sync if j % 2 == 0 else nc.scalar
                                otq = opool.tile([128, q, W], f32,
                                                 tag=f"otq{j}")
                                nc.vector.tensor_scalar_add(
                                    otq, ps[:, j * q:(j + 1) * q, :],
                                    bt[:, cot:cot + 1])
                                eng.dma_start(
                                    out=y_d[cot, :, img,
                                            blk * RB + j * q:
                                            blk * RB + (j + 1) * q, :],
                                    in_=otq)
                        else:
                            ot = opool.tile([128, RB, W], f32)
                            nc.vector.tensor_scalar_add(
                                ot, ps, bt[:, cot:cot + 1])
                            nc.sync.dma_start(
                                out=y_d[cot, :, img,
                                        blk * RB:blk * RB + RB, :], in_=ot)

    nc.compile()
    return nc


def prep_in_maps(input, weight, bias):
    """Host-side layout prep -> one in_map per core."""
    import ml_dtypes

    bf16 = ml_dtypes.bfloat16
    xp = np.pad(input, ((0, 0), (0, 0), (1, 1), (1, 1))).astype(bf16)
    # weight [co, ci, kh, kw] -> [ci, cot, (kh kw), cop]
    wr = np.ascontiguousarray(
        weight.transpose(1, 2, 3, 0).reshape(CI, KH * KW, COT, 128)
        .transpose(0, 2, 1, 3)).astype(bf16)
    b2 = np.ascontiguousarray(bias.reshape(COT, 128).T)

    in_maps = []
    for c in range(N_CORES):
        xc = np.ascontiguousarray(
            xp[c * NP_CORE:(c + 1) * NP_CORE].transpose(1, 0, 2, 3))
        in_maps.append({"x": xc, "w": wr, "b2": b2})
    return in_maps


def kernel(input, weight, bias):
    input = np.asarray(input, dtype=np.float32)
    weight = np.asarray(weight, dtype=np.float32)
    bias = np.asarray(bias, dtype=np.float32)

    if "nc" not in _CACHE:
        _CACHE["nc"] = _build_program()
    nc = _CACHE["nc"]

    from concourse.bass_utils import run_bass_kernel_spmd

    in_maps = prep_in_maps(input, weight, bias)
    res = run_bass_kernel_spmd(nc, in_maps, core_ids=list(range(N_CORES)))

    out = np.empty((N, CO, H, W), dtype=np.float32)
    for c in range(N_CORES):
        y = res.results[c]["y"]  # [COT, 128, NP_CORE, H, W]
        out[c * NP_CORE:(c + 1) * NP_CORE] = (
            y.transpose(2, 0, 1, 3, 4).reshape(NP_CORE, CO, H, W))
    return out


# revision 3
# speedup vs baseline: 1.0934x; 1.0129x over previous
"""Trainium2 Bass kernel for DenseConv2d.

Conv2d: input (32,128,56,56) f32, weight (256,128,3,3) f32, bias (256,) f32,
stride 1, pad 1, dilation 1 -> output (32,256,56,56) f32.

Strategy: data-parallel over batch across 8 NeuronCores (4 images per core).
Per core the conv is computed as accumulated matmuls (one per kernel tap)
into PSUM: out[co, pix] += W[kh,kw][ci,co].T @ x_pad[ci, shifted pix window].
Operands are cast host-side to bfloat16 so the PE array streams 1 column per
cycle (448-col matmul every ~190 ns warm) and weight loads ride the FWL fast
path; accumulation stays fp32 in PSUM.

Output rows are processed in groups per (img, cot): rows 1-8, 9-16, ...,
41-48 (8 rows, all 9 taps), rows 49-54 (6 rows), row 55 (kh=0,1 only) and
row 0 (kh=1,2 only) - the single-row edge groups skip the taps that would
only multiply the zero padding, and row 0 is processed LAST so the final
drain (6 tiny matmuls + a 1-row store) is short. Rows 49-55 share one
7-row store.

Input arrives as 10-row chunks (img 0, one per row-group, first one split
in two) interleaved across both HWDGE queues so the first group's data
lands as early as possible; weights for cot0 are split in three tap-triples
across the queues. The bias rides the gpsimd SWDGE ring to stay off the
critical queues. Images 1-3 use bigger 18-row chunks. Warmup matmuls on
scratch data bridge the DMA wait so the PE HAM clock-gate is at 2.4 GHz
when real work arrives. Layout prep (padding, channel-major transpose,
bf16 cast) is host-side numpy.
"""

import sys

if "/opt/trn_rl_repo" not in sys.path:
    sys.path.insert(0, "/opt/trn_rl_repo")

import numpy as np

N_CORES = 8
N, CI, H, W = 32, 128, 56, 56
CO, KH, KW = 256, 3, 3
NP_CORE = N // N_CORES          # images per core
HP, WP = H + 2, W + 2           # padded spatial dims
COT = CO // 128                 # out-channel tiles of 128
RB = 8                          # output rows per full matmul block
N_WARM_BIG = 6                  # 448-col warmup matmuls (~373 ns cold each)
N_WARM_TINY = 16                # 64-col warmup matmuls (~53 ns each)

# Row groups per (img, cot): (out_r0, nrows, tap positions). Taps reading
# only zero padding (kh=0 at row 0, kh=2 at row 55) are skipped. Row 0 goes
# last so the final group of the whole kernel is tiny.
ALLT = list(range(9))
GROUPS = [
    (1, 8, ALLT), (9, 8, ALLT), (17, 8, ALLT), (25, 8, ALLT),
    (33, 8, ALLT), (41, 8, ALLT), (49, 6, ALLT),
    (55, 1, [0, 1, 2, 3, 4, 5]),     # kh = 0,1
    (0, 1, [3, 4, 5, 6, 7, 8]),      # kh = 1,2
]
# img0 chunks: (padded base row, nrows); chunk k serves group k (and chunk 0
# also group 8 / chunk 6 also group 7).
CH0 = [(8 * k + 1, 10) for k in range(6)] + [(49, 8)]
G2C0 = [0, 1, 2, 3, 4, 5, 6, 6, 0]
# img 1-3 chunks (bigger):
CHI = [(1, 18), (17, 18), (33, 18), (49, 8)]
G2CI = [0, 0, 1, 1, 2, 2, 3, 3, 0]

_CACHE = {}


def _build_program():
    import concourse.mybir as mybir
    from concourse import bacc
    from concourse.tile import TileContext

    nc = bacc.Bacc(None, target_bir_lowering=False)

    bf16 = mybir.dt.bfloat16
    f32 = mybir.dt.float32

    x_d = nc.dram_tensor("x", [CI, NP_CORE, HP, WP], bf16,
                         kind="ExternalInput")
    w_d = nc.dram_tensor("w", [CI, COT, KH * KW, 128], bf16,
                         kind="ExternalInput")
    b_d = nc.dram_tensor("b2", [128, COT], f32,
                         kind="ExternalInput")
    y_d = nc.dram_tensor("y", [COT, 128, NP_CORE, H, W], f32,
                         kind="ExternalOutput")

    with TileContext(nc) as tc:
        with (
            tc.tile_pool(name="xin", bufs=1) as xpool,
            tc.tile_pool(name="wpool", bufs=1) as wpool,
            tc.tile_pool(name="bpool", bufs=1) as bpool,
            tc.tile_pool(name="psum", bufs=8, space="PSUM") as ppool,
            tc.tile_pool(name="out", bufs=6) as opool,
            tc.tile_pool(name="out7", bufs=2) as o7pool,
            tc.tile_pool(name="out1", bufs=2) as o1pool,
        ):
            # PE warmup on scratch data, concurrent with the input DMAs,
            # so the HAM clock-gate is at 2.4 GHz when real matmuls start.
            scratch = xpool.tile([CI, RB * W], bf16, tag="scratch")
            nc.vector.memset(scratch, 0.0)
            wups = ppool.tile([128, RB * W], mybir.dt.float32, tag="ps")
            for _ in range(N_WARM_BIG):
                nc.tensor.matmul(wups, scratch[:, 0:128], scratch,
                                 start=True, stop=True)
            for _ in range(N_WARM_TINY):
                nc.tensor.matmul(wups[:, 0:64], scratch[:, 0:128],
                                 scratch[:, 0:64], start=True, stop=True)

            wt = []
            for cot in range(COT):
                wtile = wpool.tile([CI, KH * KW, 128], bf16, tag=f"w{cot}")
                wt.append(wtile)
            bt = bpool.tile([128, COT], f32)

            xt = {}      # (img, chunk_idx) -> (tile, padded_base_row)

            def x_chunk_dma(img, ck, eng):
                base, rows = (CH0 if img == 0 else CHI)[ck]
                t = xpool.tile([CI, rows, WP], bf16, tag=f"x{img}_{ck}")
                eng.dma_start(out=t, in_=x_d[:, img, base:base + rows, :])
                xt[img, ck] = (t, base)

            # Critical path: the first group (img0, cot0, rows 1-8) needs
            # x rows 1-10 plus all 9 cot0 taps. Split those finely across
            # both HWDGE queues; the bias rides the gpsimd SWDGE ring.
            nc.gpsimd.dma_start(out=bt, in_=b_d[:, :])
            # x0 chunk0 in two pieces so taps kh=0 can start on rows 1-8.
            t0 = xpool.tile([CI, 10, WP], bf16, tag="x0_0")
            nc.sync.dma_start(out=t0[:, 0:8, :], in_=x_d[:, 0, 1:9, :])
            xt[0, 0] = (t0, 1)
            nc.scalar.dma_start(out=wt[0][:, 0:3, :], in_=w_d[:, 0, 0:3, :])
            nc.sync.dma_start(out=t0[:, 8:10, :], in_=x_d[:, 0, 9:11, :])
            nc.scalar.dma_start(out=wt[0][:, 3:6, :], in_=w_d[:, 0, 3:6, :])
            nc.sync.dma_start(out=wt[0][:, 6:9, :], in_=w_d[:, 0, 6:9, :])
            nc.scalar.dma_start(out=wt[1], in_=w_d[:, 1, :, :])
            for ck in range(1, len(CH0)):
                x_chunk_dma(0, ck, nc.sync)
            for img in range(1, NP_CORE):
                for ck in range(len(CHI)):
                    x_chunk_dma(img, ck, nc.scalar)

            for img in range(NP_CORE):
                g2c = G2C0 if img == 0 else G2CI
                for cot in range(COT):
                    ot7 = None
                    for gi, (r0, nr, taps) in enumerate(GROUPS):
                        ps = ppool.tile([128, RB, W], mybir.dt.float32,
                                        tag="ps")
                        xc, base = xt[img, g2c[gi]]
                        for ti, pos in enumerate(taps):
                            kh, kw = divmod(pos, KW)
                            lo = r0 + kh - base
                            rhs = xc[:, lo:lo + nr, kw:kw + W]
                            nc.tensor.matmul(
                                ps[:, 0:nr, :], wt[cot][:, pos, :], rhs,
                                start=(ti == 0), stop=(ti == len(taps) - 1),
                            )
                        if nr == RB:
                            ot = opool.tile([128, RB, W], f32)
                            nc.vector.tensor_scalar_add(
                                ot, ps, bt[:, cot:cot + 1])
                            nc.sync.dma_start(
                                out=y_d[cot, :, img, r0:r0 + RB, :], in_=ot)
                        elif nr == 6:
                            # rows 49-54: copy into the shared 7-row tile;
                            # stored together with row 55.
                            ot7 = o7pool.tile([128, 7, W], f32, tag="ot7")
                            nc.vector.tensor_scalar_add(
                                ot7[:, 0:6, :], ps[:, 0:6, :],
                                bt[:, cot:cot + 1])
                        elif r0 == H - 1:
                            nc.vector.tensor_scalar_add(
                                ot7[:, 6:7, :], ps[:, 0:1, :],
                                bt[:, cot:cot + 1])
                            nc.sync.dma_start(
                                out=y_d[cot, :, img, 49:56, :], in_=ot7)
                        else:
                            # row 0 - the last, tiny group.
                            ot1 = o1pool.tile([128, 1, W], f32, tag="ot1")
                            nc.vector.tensor_scalar_add(
                                ot1, ps[:, 0:1, :], bt[:, cot:cot + 1])
                            nc.sync.dma_start(
                                out=y_d[cot, :, img, 0:1, :], in_=ot1)

    nc.compile()
    return nc


def prep_in_maps(input, weight, bias):
    """Host-side layout prep -> one in_map per core."""
    import ml_dtypes

    bf16 = ml_dtypes.bfloat16
    xp = np.pad(input, ((0, 0), (0, 0), (1, 1), (1, 1))).astype(bf16)
    # weight [co, ci, kh, kw] -> [ci, cot, (kh kw), cop]
    wr = np.ascontiguousarray(
        weight.transpose(1, 2, 3, 0).reshape(CI, KH * KW, COT, 128)
        .transpose(0, 2, 1, 3)).astype(bf16)
    b2 = np.ascontiguousarray(bias.reshape(COT, 128).T)

    in_maps = []
    for c in range(N_CORES):
        xc = np.ascontiguousarray(
            xp[c * NP_CORE:(c + 1) * NP_CORE].transpose(1, 0, 2, 3))
        in_maps.append({"x": xc, "w": wr, "b2": b2})
    return in_maps


def kernel(input, weight, bias):
    input = np.asarray(input, dtype=np.float32)
    weight = np.asarray(weight, dtype=np.float32)
    bias = np.asarray(bias, dtype=np.float32)

    if "nc" not in _CACHE:
        _CACHE["nc"] = _build_program()
    nc = _CACHE["nc"]

    from concourse.bass_utils import run_bass_kernel_spmd

    in_maps = prep_in_maps(input, weight, bias)
    res = run_bass_kernel_spmd(nc, in_maps, core_ids=list(range(N_CORES)))

    out = np.empty((N, CO, H, W), dtype=np.float32)
    for c in range(N_CORES):
        y = res.results[c]["y"]  # [COT, 128, NP_CORE, H, W]
        out[c * NP_CORE:(c + 1) * NP_CORE] = (
            y.transpose(2, 0, 1, 3, 4).reshape(NP_CORE, CO, H, W))
    return out


# revision 5
# speedup vs baseline: 1.1007x; 1.0066x over previous
"""Trainium2 Bass kernel for DenseConv2d.

Conv2d: input (32,128,56,56) f32, weight (256,128,3,3) f32, bias (256,) f32,
stride 1, pad 1, dilation 1 -> output (32,256,56,56) f32.

Strategy: data-parallel over batch across 8 NeuronCores (4 images per core).
Per core the conv is computed as accumulated matmuls (one per kernel tap)
into PSUM: out[co, pix] += W[kh,kw][ci,co].T @ x_pad[ci, shifted pix window].
Operands are cast host-side to bfloat16 so the PE array streams 1 column per
cycle (448-col matmul every ~190 ns warm) and weight loads ride the FWL fast
path; accumulation stays fp32 in PSUM.

Output rows are processed in groups per (img, cot): rows 1-8, 9-16, ...,
41-48 (8 rows, all 9 taps), rows 49-54 (6 rows), row 55 (kh=0,1 only) and
row 0 (kh=1,2 only) - the single-row edge groups skip the taps that would
only multiply the zero padding, and row 0 is processed LAST so the final
drain (6 tiny matmuls + a 1-row store) is short. Rows 49-55 share one
7-row store.

Input arrives as 10-row chunks (img 0, one per row-group, first one split
in two) interleaved across both HWDGE queues so the first group's data
lands as early as possible; weights for cot0 are split in three tap-triples
across the queues. The bias rides the gpsimd SWDGE ring to stay off the
critical queues. Images 1-3 use bigger 18-row chunks. Warmup matmuls on
scratch data bridge the DMA wait so the PE HAM clock-gate is at 2.4 GHz
when real work arrives. Layout prep (padding, channel-major transpose,
bf16 cast) is host-side numpy.
"""

import sys

if "/opt/trn_rl_repo" not in sys.path:
    sys.path.insert(0, "/opt/trn_rl_repo")

import numpy as np

N_CORES = 8
N, CI, H, W = 32, 128, 56, 56
CO, KH, KW = 256, 3, 3
NP_CORE = N // N_CORES          # images per core
HP, WP = H + 2, W + 2           # padded spatial dims
COT = CO // 128                 # out-channel tiles of 128
RB = 8                          # output rows per full matmul block
N_WARM_BIG = 6                  # 448-col warmup matmuls (~373 ns cold each)
N_WARM_TINY = 16                # 64-col warmup matmuls (~53 ns each)

# Row groups per (img, cot): (out_r0, nrows, tap positions). Taps reading
# only zero padding (kh=0 at row 0, kh=2 at row 55) are skipped. Row 0 goes
# last so the final group of the whole kernel is tiny.
ALLT = list(range(9))
GROUPS = [
    (1, 8, ALLT), (9, 8, ALLT), (17, 8, ALLT), (25, 8, ALLT),
    (33, 8, ALLT), (41, 8, ALLT), (49, 6, ALLT),
    (55, 1, [0, 1, 2, 3, 4, 5]),     # kh = 0,1
    (0, 1, [3, 4, 5, 6, 7, 8]),      # kh = 1,2
]
# img0 chunks: (padded base row, nrows); chunk k serves group k (and chunk 0
# also group 8 / chunk 6 also group 7).
CH0 = [(8 * k + 1, 10) for k in range(6)] + [(49, 8)]
G2C0 = [0, 1, 2, 3, 4, 5, 6, 6, 0]
# img 1-3 chunks (bigger):
CHI = [(1, 18), (17, 18), (33, 18), (49, 8)]
G2CI = [0, 0, 1, 1, 2, 2, 3, 3, 0]

_CACHE = {}


def _build_program():
    import concourse.mybir as mybir
    from concourse import bacc
    from concourse.tile import TileContext

    nc = bacc.Bacc(None, target_bir_lowering=False)

    bf16 = mybir.dt.bfloat16
    f32 = mybir.dt.float32

    x_d = nc.dram_tensor("x", [CI, NP_CORE, HP, WP], bf16,
                         kind="ExternalInput")
    w_d = nc.dram_tensor("w", [CI, COT, KH * KW, 128], bf16,
                         kind="ExternalInput")
    b_d = nc.dram_tensor("b2", [128, COT], f32,
                         kind="ExternalInput")
    y_d = nc.dram_tensor("y", [COT, 128, NP_CORE, H, W], f32,
                         kind="ExternalOutput")

    with TileContext(nc) as tc:
        with (
            tc.tile_pool(name="xin", bufs=1) as xpool,
            tc.tile_pool(name="wpool", bufs=1) as wpool,
            tc.tile_pool(name="bpool", bufs=1) as bpool,
            tc.tile_pool(name="psum", bufs=8, space="PSUM") as ppool,
            tc.tile_pool(name="out", bufs=6) as opool,
            tc.tile_pool(name="out7", bufs=2) as o7pool,
            tc.tile_pool(name="out1", bufs=2) as o1pool,
        ):
            # PE warmup on scratch data, concurrent with the input DMAs,
            # so the HAM clock-gate is at 2.4 GHz when real matmuls start.
            scratch = xpool.tile([CI, RB * W], bf16, tag="scratch")
            nc.vector.memset(scratch, 0.0)
            wups = ppool.tile([128, RB * W], mybir.dt.float32, tag="ps")
            for _ in range(N_WARM_BIG):
                nc.tensor.matmul(wups, scratch[:, 0:128], scratch,
                                 start=True, stop=True)
            for _ in range(N_WARM_TINY):
                nc.tensor.matmul(wups[:, 0:64], scratch[:, 0:128],
                                 scratch[:, 0:64], start=True, stop=True)

            wt = []
            for cot in range(COT):
                wtile = wpool.tile([CI, KH * KW, 128], bf16, tag=f"w{cot}")
                wt.append(wtile)
            bt = bpool.tile([128, COT], f32)

            xt = {}      # (img, chunk_idx) -> (tile, padded_base_row)

            def x_chunk_dma(img, ck, eng):
                base, rows = (CH0 if img == 0 else CHI)[ck]
                t = xpool.tile([CI, rows, WP], bf16, tag=f"x{img}_{ck}")
                eng.dma_start(out=t, in_=x_d[:, img, base:base + rows, :])
                xt[img, ck] = (t, base)

            # Critical path: the first group (img0, cot0, rows 1-8) needs
            # x rows 1-10 plus all 9 cot0 taps. Split those finely across
            # both HWDGE queues; the bias rides the gpsimd SWDGE ring.
            nc.gpsimd.dma_start(out=bt, in_=b_d[:, :])
            # x0 chunk0 in two pieces so taps kh=0 can start on rows 1-8.
            t0 = xpool.tile([CI, 10, WP], bf16, tag="x0_0")
            nc.sync.dma_start(out=t0[:, 0:8, :], in_=x_d[:, 0, 1:9, :])
            xt[0, 0] = (t0, 1)
            nc.scalar.dma_start(out=wt[0][:, 0:3, :], in_=w_d[:, 0, 0:3, :])
            nc.sync.dma_start(out=t0[:, 8:10, :], in_=x_d[:, 0, 9:11, :])
            nc.scalar.dma_start(out=wt[0][:, 3:6, :], in_=w_d[:, 0, 3:6, :])
            nc.sync.dma_start(out=wt[0][:, 6:9, :], in_=w_d[:, 0, 6:9, :])
            # img0 chunks alternate queues so neither ring backs up; w1 and
            # the img1-3 chunks follow on the scalar ring (needed late).
            x_chunk_dma(0, 1, nc.scalar)
            x_chunk_dma(0, 2, nc.sync)
            x_chunk_dma(0, 3, nc.scalar)
            x_chunk_dma(0, 4, nc.sync)
            x_chunk_dma(0, 5, nc.scalar)
            x_chunk_dma(0, 6, nc.sync)
            nc.scalar.dma_start(out=wt[1], in_=w_d[:, 1, :, :])
            for img in range(1, NP_CORE):
                for ck in range(len(CHI)):
                    x_chunk_dma(img, ck, nc.scalar)

            for img in range(NP_CORE):
                g2c = G2C0 if img == 0 else G2CI
                for cot in range(COT):
                    ot7 = None
                    for gi, (r0, nr, taps) in enumerate(GROUPS):
                        ps = ppool.tile([128, RB, W], mybir.dt.float32,
                                        tag="ps")
                        xc, base = xt[img, g2c[gi]]
                        for ti, pos in enumerate(taps):
                            kh, kw = divmod(pos, KW)
                            lo = r0 + kh - base
                            rhs = xc[:, lo:lo + nr, kw:kw + W]
                            nc.tensor.matmul(
                                ps[:, 0:nr, :], wt[cot][:, pos, :], rhs,
                                start=(ti == 0), stop=(ti == len(taps) - 1),
                            )
                        if nr == RB:
                            ot = opool.tile([128, RB, W], f32)
                            nc.vector.tensor_scalar_add(
                                ot, ps, bt[:, cot:cot + 1])
                            nc.sync.dma_start(
                                out=y_d[cot, :, img, r0:r0 + RB, :], in_=ot)
                        elif nr == 6:
                            # rows 49-54: copy into the shared 7-row tile;
                            # stored together with row 55.
                            ot7 = o7pool.tile([128, 7, W], f32, tag="ot7")
                            nc.vector.tensor_scalar_add(
                                ot7[:, 0:6, :], ps[:, 0:6, :],
                                bt[:, cot:cot + 1])
                        elif r0 == H - 1:
                            nc.vector.tensor_scalar_add(
                                ot7[:, 6:7, :], ps[:, 0:1, :],
                                bt[:, cot:cot + 1])
                            nc.sync.dma_start(
                                out=y_d[cot, :, img, 49:56, :], in_=ot7)
                        else:
                            # row 0 - the last, tiny group. Its store rides
                            # the scalar ring (drained by then) so it does
                            # not queue behind the 7-row store's issue.
                            ot1 = o1pool.tile([128, 1, W], f32, tag="ot1")
                            nc.vector.tensor_scalar_add(
                                ot1, ps[:, 0:1, :], bt[:, cot:cot + 1])
                            nc.scalar.dma_start(
                                out=y_d[cot, :, img, 0:1, :], in_=ot1)

    nc.compile()
    return nc


def prep_in_maps(input, weight, bias):
    """Host-side layout prep -> one in_map per core."""
    import ml_dtypes

    bf16 = ml_dtypes.bfloat16
    xp = np.pad(input, ((0, 0), (0, 0), (1, 1), (1, 1))).astype(bf16)
    # weight [co, ci, kh, kw] -> [ci, cot, (kh kw), cop]
    wr = np.ascontiguousarray(
        weight.transpose(1, 2, 3, 0).reshape(CI, KH * KW, COT, 128)
        .transpose(0, 2, 1, 3)).astype(bf16)
    b2 = np.ascontiguousarray(bias.reshape(COT, 128).T)

    in_maps = []
    for c in range(N_CORES):
        xc = np.ascontiguousarray(
            xp[c * NP_CORE:(c + 1) * NP_CORE].transpose(1, 0, 2, 3))
        in_maps.append({"x": xc, "w": wr, "b2": b2})
    return in_maps


def kernel(input, weight, bias):
    input = np.asarray(input, dtype=np.float32)
    weight = np.asarray(weight, dtype=np.float32)
    bias = np.asarray(bias, dtype=np.float32)

    if "nc" not in _CACHE:
        _CACHE["nc"] = _build_program()
    nc = _CACHE["nc"]

    from concourse.bass_utils import run_bass_kernel_spmd

    in_maps = prep_in_maps(input, weight, bias)
    res = run_bass_kernel_spmd(nc, in_maps, core_ids=list(range(N_CORES)))

    out = np.empty((N, CO, H, W), dtype=np.float32)
    for c in range(N_CORES):
        y = res.results[c]["y"]  # [COT, 128, NP_CORE, H, W]
        out[c * NP_CORE:(c + 1) * NP_CORE] = (
            y.transpose(2, 0, 1, 3, 4).reshape(NP_CORE, CO, H, W))
    return out
